# revision 40
# baseline (speedup 1.0000x reference)
"""GLIFR RNN (nn_BNNFC) Trainium2 Bass kernel — 8-core batch-data-parallel.

Strategy
--------
- Batch (64) sharded 8 ways -> 8 batch elements per core; weights replicated.
- The 20-step synaptic delay means the lateral matmul input firing(t-20) is
  known a whole block of 20 steps in advance, so lateral/input/readout
  matmuls run as batched [*, (t,b)] matmuls per 20-step block on TensorE.
- Only the elementwise state recurrence (asc currents, voltage, sigmoid) is
  truly sequential: 8 VectorE ops + 1 ScalarE sigmoid + 3 GPSIMD ops per
  step on [128, (h_outer=8, b=8)] tiles (H=1024 split as h = j*128 + p),
  refactored so only mul+add+sigmoid sit on the step-to-step chain.
- All rate constants are folded host-side:
    sg = sigmoid(trans_k_m); c1 = R*sg; c2 = 1-sg
    W_in' = W_in*c1, W_lat' = W_lat*c1 (column-scaled)
    A := c1*asc  =>  A(t) = (p*u+q)*A(t-1) + s'*u,  p=r*dka, q=1-dka,
    s' = c1*dka*amp;  vs := volt-thresh:
    vs(t) = syn'(t) + A1(t)+A2(t) + c2*vs(t-1),  syn' = c1*syn - sg*thresh
    firing(t) = sigmoid(vs(t))
"""

import os
import numpy as np
import ml_dtypes

import concourse.bacc as bacc
from concourse.tile import add_dep_helper
import concourse.tile as tile
import concourse.mybir as mybir
from concourse.bass_utils import run_bass_kernel_spmd

# problem constants
B, T, IN, HID, OUT = 64, 200, 512, 1024, 512
DELAY, NA = 20, 2
R_MEM = 0.1
N_CORES = 8
BC = B // N_CORES            # 8 batch per core
J = HID // 128               # 8 hidden chunks
KCI = IN // 128              # 4 input contraction chunks
OC = OUT // 128              # 4 output chunks
NBLK = T // DELAY            # 10 blocks of 20 steps
TB = DELAY                   # steps per block

MM_DT_S = os.environ.get("GLIFR_MM_DT", "bf16")   # matmul operand dtype
EW_DT_S = os.environ.get("GLIFR_EW_DT", "bf16")   # elementwise state dtype
ABLATE = os.environ.get("GLIFR_ABLATE", "")       # dev-only timing bisect

_DT = {"f32": mybir.dt.float32, "bf16": mybir.dt.bfloat16}
_NP = {"f32": np.float32, "bf16": ml_dtypes.bfloat16}

_CACHE = {}


def _build(mm_s, ew_s):
    mm = _DT[mm_s]
    ew = _DT[ew_s]
    f32 = mybir.dt.float32
    Act = mybir.ActivationFunctionType

    nc = bacc.Bacc("TRN2", target_bir_lowering=False, debug=False,
                   num_devices=N_CORES)

    # ---- DRAM parameters (per-core) ----
    d_xT = nc.dram_tensor("xT", [KCI, 128, T, BC], mm, kind="ExternalInput")
    d_win = nc.dram_tensor("w_in", [KCI, 128, HID], mm, kind="ExternalInput")
    d_wlat = nc.dram_tensor("w_lat", [J, 128, HID], mm, kind="ExternalInput")
    d_wout = nc.dram_tensor("w_out", [J, 128, OUT], mm, kind="ExternalInput")
    # all small elementwise constants packed into two tensors (one DMA each):
    # cb = [cP | cQ | cS | cC2 | d1_0] along the free axis, cf = [biasx|bout]
    d_cb = nc.dram_tensor("cb", [128, 3 * NA * J * BC + 3 * J * BC], ew,
                          kind="ExternalInput")
    d_cf = nc.dram_tensor("cf", [128, J], f32, kind="ExternalInput")
    d_out = nc.dram_tensor("outT", [OC, 128, T, BC], f32, kind="ExternalOutput")

    HB = TB // 2   # lateral half-block = 10 steps

    with tile.TileContext(nc) as tc:
        with (
            tc.tile_pool(name="weights", bufs=1) as wpool,
            tc.tile_pool(name="state", bufs=1) as spool,
            tc.tile_pool(name="ew", bufs=2) as epool,
            tc.tile_pool(name="synp", bufs=2) as synpool,
            tc.tile_pool(name="ost", bufs=2) as opool,
            tc.tile_pool(name="ps_xp", bufs=1, space="PSUM") as psxp,
            tc.tile_pool(name="ps_lat", bufs=1, space="PSUM") as pslat,
            tc.tile_pool(name="ps_ro", bufs=1, space="PSUM") as psro,
        ):
            # ---- persistent tiles ----
            t_win = wpool.tile([128, KCI, HID], mm, tag="win")
            t_wlat = wpool.tile([128, J, HID], mm, tag="wlat")
            t_wout = wpool.tile([128, J, OUT], mm, tag="wout")
            # packed constants (one DMA each); shaped views below
            NAJB = NA * J * BC
            JB = J * BC
            t_cb = wpool.tile([128, 3 * NAJB + 3 * JB], ew, tag="cb")
            t_cf = wpool.tile([128, J], f32, tag="cf")
            def cb3(i):
                return t_cb[:, i * NAJB:(i + 1) * NAJB].rearrange(
                    "p (a j b) -> p a j b", a=NA, j=J)

            def cb1(i):
                return t_cb[:, 3 * NAJB + i * JB:3 * NAJB + (i + 1) * JB] \
                    .rearrange("p (j b) -> p j b", j=J)

            ap_cP, ap_cQ, ap_cS = cb3(0), cb3(1), cb3(2)
            ap_cC2, ap_d1init, ap_cbx = cb1(0), cb1(1), cb1(2)

            def ap_biasx(j):
                return t_cf[:, j:j + 1]

            # x split head/rest: tile-granular deps mean block-0 matmuls
            # must not share a tile with the bulk transfer
            t_xTh = wpool.tile([128, KCI, TB, BC], mm, tag="xTh")
            t_xTr = wpool.tile([128, KCI, T - TB, BC], mm, tag="xTr")
            # Block 0 has no lateral input (delay buffer is zeros), so its
            # synaptic drive is just c1*x@W_in - sg*thresh, staged here. For
            # blocks >= 1 the x-projection accumulates directly into the
            # lateral PSUM banks (emit_group) and never lands in SBUF
            # separately.
            t_xsyn0a = wpool.tile([128, J, 4, BC], mybir.dt.bfloat16,
                                  tag="xsyn0a")
            t_xsyn0b = wpool.tile([128, J, TB - 4, BC], mybir.dt.bfloat16,
                                  tag="xsyn0b")

            # F_buf slot s holds firing(s-1); slot 0 = zeros
            t_F = spool.tile([128, J, T + 1, BC], mm, tag="F")
            t_A = spool.tile([128, NA, J, BC], ew, tag="A")
            t_Aq = spool.tile([128, NA, J, BC], ew, tag="Aq")
            t_Bst = spool.tile([128, NA, J, BC], ew, tag="Bst")
            t_vs = [spool.tile([128, J, BC], ew, tag=f"vs{i}", name=f"vs{i}")
                    for i in range(2)]
            t_D = [spool.tile([128, J, BC], ew, tag=f"D{i}", name=f"D{i}")
                    for i in range(2)]

            # ---- act-table preload + input DMAs ----
            # Dummy sigmoid on a zeroed scratch forces the 1.3us activation
            # table load to overlap the input DMAs instead of serializing
            # before the first real activation.
            t_scr = wpool.tile([128, 1], ew, tag="scr")
            nc.vector.memset(t_scr[:], 0.0)
            nc.scalar.activation(out=t_scr[:], in_=t_scr[:], func=Act.Sigmoid)
            # DMA order mirrors first-use time; the model serializes all
            # transfers, so the tiny constants must not queue behind bulk.
            nc.sync.dma_start(out=t_cb[:], in_=d_cb.ap())
            nc.sync.dma_start(out=t_cf[:], in_=d_cf.ap())
            nc.sync.dma_start(out=t_win[:],
                              in_=d_win.ap().rearrange("k p h -> p k h"))
            nc.scalar.dma_start(out=t_xTh[:],
                                in_=d_xT.ap()[:, :, 0:TB, :]
                                    .rearrange("k p t b -> p k t b"))
            nc.sync.dma_start(out=t_wlat[:],
                              in_=d_wlat.ap().rearrange("k p h -> p k h"))
            nc.sync.dma_start(out=t_xTr[:],
                              in_=d_xT.ap()[:, :, TB:T, :]
                                  .rearrange("k p t b -> p k t b"))
            nc.sync.dma_start(out=t_wout[:],
                              in_=d_wout.ap().rearrange("k p o -> p k o"))

            # ---- state init ----
            nc.vector.memset(t_Aq[:], 0.0)
            nc.vector.memset(t_Bst[:], 0.0)   # u(0)=0 makes any Bst(-1) ok
            nc.vector.memset(t_F[:, :, 0, :], 0.0)

            # block-0 xproj: two chunks, each all-j in one psum tile and
            # ONE bias-free ACT copy (the -sg*thresh bias is applied by an
            # extra Pool add in block 0's D-chain and folded into D(-1)).
            XCHUNKS0 = [(0, 4), (4, 16)]

            def emit_xproj_chunk(tci, dst):
                lo, ln = XCHUNKS0[tci]
                if "no_mm" in ABLATE:
                    nc.gpsimd.memset(dst[:], 0.0)
                    return
                ps = psxp.tile([128, J, 16, BC], f32, tag="xp0")
                for j in range(J):
                    for kc in range(KCI):
                        nc.tensor.matmul(
                            out=ps[:, j, 0:ln, :],
                            lhsT=t_win[:, kc, j * 128:(j + 1) * 128],
                            rhs=t_xTh[:, kc, lo:lo + ln, :],
                            start=(kc == 0), stop=(kc == KCI - 1))
                nc.scalar.activation(
                    out=dst[:], in_=ps[:, :, 0:ln, :], func=Act.Identity,
                    scale=1.0)

            def emit_group(k, ps, j, a, b):
                """syn psum group for block k (>=1), chunk j, steps [a,b) of
                the block: xproj matmuls open the accumulation group, the
                lateral matmuls accumulate on top. The lateral input is
                firing(k*TB+a-20 .. ), i.e. F slots (k-1)*TB+1+a, so the
                group is runnable once sigma(k*TB+a+b-TB-? ) -- in practice
                once sigma((k-1)*TB + b - 1) of the previous block fired."""
                if "no_mm" in ABLATE:
                    return
                ln = b - a
                out = ps[:, j, a * BC:b * BC].rearrange(
                    "p (t b) -> p t b", t=ln)
                x0 = (k - 1) * TB + a   # t_xTr starts at t=TB; k >= 1 here
                for kc in range(KCI):
                    nc.tensor.matmul(
                        out=out,
                        lhsT=t_win[:, kc, j * 128:(j + 1) * 128],
                        rhs=t_xTr[:, kc, x0:x0 + ln, :],
                        start=(kc == 0),
                        stop=(kc == KCI - 1 and "no_lat" in ABLATE))
                if "no_lat" in ABLATE:
                    return
                s0 = (k - 1) * TB + 1 + a
                for kc in range(J):
                    nc.tensor.matmul(
                        out=out,
                        lhsT=t_wlat[:, kc, j * 128:(j + 1) * 128],
                        rhs=t_F[:, kc, s0:s0 + ln, :],
                        start=False, stop=(kc == J - 1))

            def emit_syn_copy(k, ps, syn, j, a, b, pin_t=None):
                """syn_sb[j, a:b] = psum (xproj + lateral) + bias, one ACT
                copy; the per-hidden bias rides the activation's bias port.
                Pinned after the sigmoid one step past the group's last
                lateral input so it never stalls the sigmoid cadence."""
                if "no_mm" in ABLATE:
                    nc.gpsimd.memset(syn[:, j, a:b, :], 0.0)
                    return
                ci = nc.scalar.activation(
                    out=syn[:, j, a:b, :],
                    in_=ps[:, j, a * BC:b * BC].rearrange(
                        "p (t b) -> p t b", t=b - a),
                    func=Act.Identity, bias=ap_biasx(j), scale=1.0)
                pin_after_sigma(ci, pin_t if pin_t is not None
                                else (k - 1) * TB + b - 1)

            def emit_ro(k, deferred, defer_map=None, pieces=4):
                """readout for block k in `pieces` t-slices; each slice is
                4oc x 8kc matmuls + ONE bias-free store copy (all oc) + ONE
                DMA. b_out is added host-side after the gather. With
                defer_map, slice q's work goes to defer_map[q] so only the
                last slice trails the final sigmoid."""
                if "no_mm" in ABLATE or "no_ro" in ABLATE:
                    return
                QL = TB // pieces
                ps = psro.tile([128, OC, 256], f32, tag="ro")
                for q in range(pieces):
                    tgt = deferred if defer_map is None else defer_map[q]
                    s0 = k * TB + 1 + q * QL
                    for oc in range(OC):
                        for kc in range(J):
                            tgt.append(
                                lambda oc=oc, kc=kc, q=q, s0=s0, ps=ps:
                                nc.tensor.matmul(
                                    out=ps[:, oc,
                                           q * QL * BC:(q + 1) * QL * BC]
                                    .rearrange("p (t b) -> p t b", t=QL),
                                    lhsT=t_wout[:, kc,
                                                oc * 128:(oc + 1) * 128],
                                    rhs=t_F[:, kc, s0:s0 + QL, :],
                                    start=(kc == 0), stop=(kc == J - 1)))

                    def flush(q=q, ps=ps, s0=s0):
                        ot = opool.tile([128, OC, QL, BC], f32, tag="ost")
                        ci = nc.scalar.activation(
                            out=ot[:],
                            in_=ps[:, :, q * QL * BC:(q + 1) * QL * BC]
                            .rearrange("p o (t b) -> p o t b", t=QL),
                            func=Act.Identity, scale=1.0)
                        pin_after_sigma(ci, s0 + QL + 2 * q)
                        nc.sync.dma_start(
                            out=d_out.ap()[:, :,
                                           k * TB + q * QL:
                                           k * TB + (q + 1) * QL, :]
                            .rearrange("o p t b -> p o t b"),
                            in_=ot[:])
                    tgt.append(flush)

            sig_insts = {}

            def pin_after_sigma(inst, t):
                """Order `inst` after sigma(t): keeps the scheduler from
                slotting PSUM-copy ops into the ACT stream before their
                matmul group has finished, which stalls later sigmoids."""
                si = sig_insts.get(min(t, T - 1))
                if si is not None and inst is not None:
                    add_dep_helper(inst.ins, si.ins,
                                   reason="pin copy after sigma")

            def emit_ew_step(t, syn, syn_funcs):
                """B-form recurrence step; reads F slot t, writes slot t+1.

                Critical path after sigma(t-1): mb2 -> msum -> vs -> sigma(t).
                Everything else overlaps the ScalarE sigmoid round-trip; the
                d1 update runs on the GPSIMD engine. vs and d1 are
                double-buffered (t%2) to break cross-engine WAR stalls.
                """
                if "no_ew" in ABLATE:
                    return
                cur, prv = t % 2, (t + 1) % 2
                u2 = t_F[:, :, t, :].unsqueeze(1) \
                    .broadcast_to([128, NA, J, BC])
                # critical: vs(t) = u(t)*(B0+B1)(t-1) + D(t-1)
                mb2 = epool.tile([128, NA, J, BC], ew, tag="mb2")
                msum = epool.tile([128, J, BC], ew, tag="msum")
                with tc.high_priority(offset=40):
                    nc.vector.tensor_mul(out=mb2[:], in0=u2, in1=t_Bst[:])
                    nc.vector.tensor_add(out=msum[:], in0=mb2[:, 0],
                                         in1=mb2[:, 1])
                    nc.vector.tensor_add(out=t_vs[cur][:], in0=msum[:],
                                         in1=t_D[prv][:])
                    if "no_sigma" not in ABLATE:
                        sig_insts[t] = nc.scalar.activation(
                            out=t_F[:, :, t + 1, :],
                            in_=t_vs[cur][:], func=Act.Sigmoid)
                # state updates (overlap sigma): A(t) = Aq(t-1) + mb2
                nc.vector.tensor_add(out=t_A[:], in0=t_Aq[:], in1=mb2[:])
                nc.vector.tensor_mul(out=t_Aq[:], in0=t_A[:], in1=ap_cQ)
                qa = epool.tile([128, J, BC], ew, tag="qa")
                nc.vector.tensor_add(out=qa[:], in0=t_Aq[:, 0], in1=t_Aq[:, 1])
                # B(t) = p*A(t) + s'   (on the step loop -> keep on DVE)
                bp = epool.tile([128, NA, J, BC], ew, tag="bp")
                nc.vector.tensor_mul(out=bp[:], in0=t_A[:], in1=ap_cP)
                nc.vector.tensor_add(out=t_Bst[:], in0=bp[:], in1=ap_cS)
                # D(t) = qa + syn'(t+1) + c2*vs(t) on Pool. d1 = qa+syn does
                # not depend on vs, so only cv -> D trails the vs write; D
                # lands ~vs+650 instead of ~vs+1100 (3-op serial chain).
                sc = epool.tile([128, J, BC], ew, tag="scv")
                cv = epool.tile([128, J, BC], ew, tag="cv")
                if t + 1 < T:
                    nxt = syn_funcs[(t + 1) // TB]
                    # D = (c2*vs + syn'(t+1)) + qa, assembled so only the
                    # final add trails qa (the last-arriving operand): the
                    # qa -> D path is one Pool op instead of two.
                    nc.gpsimd.tensor_mul(out=cv[:], in0=t_vs[cur][:],
                                         in1=ap_cC2)
                    nc.gpsimd.tensor_add(out=sc[:], in0=cv[:], in1=nxt(t + 1))
                    if t + 1 < TB:
                        # block-0 syn slices are bias-free (batched copies):
                        # apply -sg*thresh here
                        sb = epool.tile([128, J, BC], ew, tag="scb")
                        nc.gpsimd.tensor_add(out=sb[:], in0=sc[:], in1=ap_cbx)
                        sc = sb
                    nc.gpsimd.tensor_add(out=t_D[cur][:], in0=sc[:],
                                         in1=qa[:])

            # ---------- main schedule ----------
            # Block-0 input projection upfront. D(-1) init only needs the
            # first 4-step chunk (separate tile -> no false dep on chunk 2).
            emit_xproj_chunk(0, t_xsyn0a)
            nc.gpsimd.tensor_add(out=t_D[1][:], in0=ap_d1init,
                                 in1=t_xsyn0a[:, :, 0, :])
            emit_xproj_chunk(1, t_xsyn0b)

            def xsyn0_slice(t):
                if t < 4:
                    return t_xsyn0a[:, :, t, :]
                return t_xsyn0b[:, :, t - 4, :]

            def synsb_slice(syn):
                def f(t):
                    return syn[:, :, t % TB, :]
                return f


            syn_funcs = {0: xsyn0_slice}     # block 0 reads xsyn0 directly
            ps_next = None
            syn_next = None
            defC = []   # this block's half-1 syn groups (data complete at
                        # the preceding block boundary); popped steps 0..8
            for k in range(NBLK):
                # defA: popped during EW steps 0..8 after defC: block k-1's
                #   readout.
                # defB: popped during EW steps 10..18: block k+1 syn psum
                #   groups, half 0 (xproj + lateral, consuming this block's
                #   first-half firing as it appears) + their syn copies.
                defA, defB, defQ = list(defC), [], []
                defC = []
                if k >= 1:
                    emit_ro(k - 1, defA)
                if k + 1 < NBLK:
                    ps_next = pslat.tile([128, J, 256], f32, tag="lat")
                    syn_next = synpool.tile([128, J, TB, BC], ew, tag="syn_sb")
                    syn_funcs[k + 1] = synsb_slice(syn_next)
                    for j in range(J):
                        # half 0 (steps 0..9 of block k+1): lateral input is
                        # firing from this block's steps 1..10 -> runnable
                        # mid-block; popped steps 10..14.
                        defB.append(lambda j=j, ps=ps_next, kk=k + 1:
                                    emit_group(kk, ps, j, 0, HB))
                        defB.append(lambda j=j, ps=ps_next, sy=syn_next,
                                    kk=k + 1, pt=k * TB + 11 + j // 2:
                                    emit_syn_copy(kk, ps, sy, j, 0, HB,
                                                  pin_t=pt))
                        # quarter q2 (steps 10..14): needs sigma(step 14) of
                        # this block -> drains during this block's tail
                        # (groups only; the copy waits for q3).
                        defQ.append(lambda j=j, ps=ps_next, kk=k + 1:
                                    emit_group(kk, ps, j, HB, HB + 5))
                        # quarter q3 (steps 15..19): needs this block's last
                        # sigmoid; popped during EW(k+1) steps 0..8. One
                        # copy then covers the whole half [10,20), landing
                        # before E(step 9) consumes syn(10).
                        defC.append(lambda j=j, ps=ps_next, kk=k + 1:
                                    emit_group(kk, ps, j, HB + 5, TB))
                        defC.append(lambda j=j, ps=ps_next, sy=syn_next,
                                    kk=k + 1, pt=(k + 1) * TB + j + 1:
                                    emit_syn_copy(kk, ps, sy, j, HB, TB,
                                                  pin_t=pt))
                if k == NBLK - 1:
                    ro_tail = []
                    # quarters q0/q1 during steps 10-14, q2 during 15-18,
                    # q3 (needs the last sigmoid) in the tail
                    emit_ro(NBLK - 1, None, pieces=4,
                            defer_map={0: defB, 1: defB, 2: defQ,
                                       3: ro_tail})

                perA = max(1, (len(defA) + 8) // 9)
                perB = max(1, (len(defB) + 4) // 5)
                perQ = max(1, (len(defQ) + 3) // 4)
                for li, t in enumerate(range(k * TB, (k + 1) * TB)):
                    emit_ew_step(t, syn_funcs[k], syn_funcs)
                    if li < 10:
                        pend, per = defA, perA
                    elif li < 15:
                        pend, per = defB, perB
                    else:
                        pend, per = defQ, perQ
                    for _ in range(per):
                        if pend:
                            pend.pop(0)()
                for fn in defA + defB + defQ:
                    fn()

            # final readout tail (half 1 + stores; half 0 ran in EW(9))
            for fn in ro_tail:
                fn()

    nc.compile()
    return nc


def _sigmoid(x):
    return 1.0 / (1.0 + np.exp(-x))


def _prep(inputs, mm_s, ew_s):
    mmn = _NP[mm_s]
    ewn = _NP[ew_s]
    f32 = np.float32

    x = np.asarray(inputs["x"], f32)
    W_in = np.asarray(inputs["W_in"], f32)
    W_lat = np.asarray(inputs["W_lat"], f32)
    thresh = np.asarray(inputs["thresh"], f32)[0]
    trans_k_m = np.asarray(inputs["trans_k_m"], f32)[0]
    trans_asc_k = np.asarray(inputs["trans_asc_k"], f32)[:, 0, :]
    asc_amp = np.asarray(inputs["asc_amp"], f32)[:, 0, :]
    trans_asc_r = np.asarray(inputs["trans_asc_r"], f32)[:, 0, :]
    W_out = np.asarray(inputs["W_out"], f32)
    b_out = np.asarray(inputs["b_out"], f32)

    sg = _sigmoid(trans_k_m).astype(f32)
    c1 = (R_MEM * sg).astype(f32)
    c2 = (1.0 - sg).astype(f32)
    dka = _sigmoid(trans_asc_k).astype(f32)
    r_a = (1.0 - 2.0 * _sigmoid(trans_asc_r)).astype(f32)
    p_a = (r_a * dka).astype(f32)
    q_a = (1.0 - dka).astype(f32)
    s_a = (c1[None] * dka * asc_amp).astype(f32)
    bias_h = (-sg * thresh).astype(f32)

    w_in = (W_in * c1[None, :]).astype(mmn).reshape(KCI, 128, HID)
    w_lat = (W_lat * c1[None, :]).astype(mmn).reshape(J, 128, HID)
    w_out = np.ascontiguousarray(W_out.T).astype(mmn).reshape(J, 128, OUT)

    def hb(coef_ah):  # [NA,H] -> [128, NA, J, BC]
        a = coef_ah.reshape(NA, J, 128).transpose(2, 0, 1)
        return np.broadcast_to(a[..., None], (128, NA, J, BC)).astype(ewn).copy()

    def hb1(coef_h):  # [H] -> [128, J, BC]
        a = coef_h.reshape(J, 128).T
        return np.broadcast_to(a[..., None], (128, J, BC)).astype(ewn).copy()

    cP, cQ, cS = hb(p_a), hb(q_a), hb(s_a)
    cC2 = hb1(c2)
    # block-0 syn slices carry no bias, so D(-1) folds it in: -(c2+sg)*th
    d1_0 = hb1((-(c2 + sg) * thresh).astype(f32))
    cbx = hb1(bias_h)
    bias_x = np.ascontiguousarray(bias_h.reshape(J, 128).T).astype(f32)
    b_outT = np.ascontiguousarray(b_out.reshape(OC, 128).T).astype(f32)

    # packed constants: cb = [cP|cQ|cS|cC2|d1_0|cbx], cf = biasx
    cb = np.concatenate(
        [a.reshape(128, -1) for a in (cP, cQ, cS, cC2, d1_0, cbx)],
        axis=1).astype(ewn)
    cf = bias_x

    in_maps = []
    for c in range(N_CORES):
        xc = x[c * BC:(c + 1) * BC]                    # [8, 200, 512]
        xT = np.ascontiguousarray(xc.transpose(2, 1, 0)).astype(mmn) \
            .reshape(KCI, 128, T, BC)
        in_maps.append({
            "xT": xT, "w_in": w_in, "w_lat": w_lat, "w_out": w_out,
            "cb": cb, "cf": cf,
        })
    return in_maps


def _get_nc():
    key = (MM_DT_S, EW_DT_S, ABLATE)
    if key not in _CACHE:
        _CACHE[key] = _build(MM_DT_S, EW_DT_S)
    return _CACHE[key]


def kernel(**inputs) -> np.ndarray:
    nc = _get_nc()
    in_maps = _prep(inputs, MM_DT_S, EW_DT_S)
    try:
        res = run_bass_kernel_spmd(nc, in_maps, list(range(N_CORES)))
    except Exception:
        # transient NRT device errors have been observed through the axon
        # tunnel; one retry normally succeeds
        import time as _time
        _time.sleep(2.0)
        res = run_bass_kernel_spmd(nc, in_maps, list(range(N_CORES)))
    out = np.empty((B, T, OUT), np.float32)
    for c in range(N_CORES):
        r = res.results[c]["outT"]                     # [OC, 128, T, BC]
        out[c * BC:(c + 1) * BC] = r.transpose(3, 2, 0, 1).reshape(BC, T, OUT)
    out += np.asarray(inputs["b_out"], np.float32)  # bias applied host-side
    return out



# revision 41
# speedup vs baseline: 1.0067x; 1.0067x over previous
"""GLIFR RNN (nn_BNNFC) Trainium2 Bass kernel — 8-core batch-data-parallel.

Strategy
--------
- Batch (64) sharded 8 ways -> 8 batch elements per core; weights replicated.
- The 20-step synaptic delay means the lateral matmul input firing(t-20) is
  known a whole block of 20 steps in advance, so lateral/input/readout
  matmuls run as batched [*, (t,b)] matmuls per 20-step block on TensorE.
- Only the elementwise state recurrence (asc currents, voltage, sigmoid) is
  truly sequential: 8 VectorE ops + 1 ScalarE sigmoid + 3 GPSIMD ops per
  step on [128, (h_outer=8, b=8)] tiles (H=1024 split as h = j*128 + p),
  refactored so only mul+add+sigmoid sit on the step-to-step chain.
- All rate constants are folded host-side:
    sg = sigmoid(trans_k_m); c1 = R*sg; c2 = 1-sg
    W_in' = W_in*c1, W_lat' = W_lat*c1 (column-scaled)
    A := c1*asc  =>  A(t) = (p*u+q)*A(t-1) + s'*u,  p=r*dka, q=1-dka,
    s' = c1*dka*amp;  vs := volt-thresh:
    vs(t) = syn'(t) + A1(t)+A2(t) + c2*vs(t-1),  syn' = c1*syn - sg*thresh
    firing(t) = sigmoid(vs(t))
"""

import os
import numpy as np
import ml_dtypes

import concourse.bacc as bacc
from concourse.tile import add_dep_helper
import concourse.tile as tile
import concourse.mybir as mybir
from concourse.bass_utils import run_bass_kernel_spmd

# problem constants
B, T, IN, HID, OUT = 64, 200, 512, 1024, 512
DELAY, NA = 20, 2
R_MEM = 0.1
N_CORES = 8
BC = B // N_CORES            # 8 batch per core
J = HID // 128               # 8 hidden chunks
KCI = IN // 128              # 4 input contraction chunks
OC = OUT // 128              # 4 output chunks
NBLK = T // DELAY            # 10 blocks of 20 steps
TB = DELAY                   # steps per block

MM_DT_S = os.environ.get("GLIFR_MM_DT", "bf16")   # matmul operand dtype
EW_DT_S = os.environ.get("GLIFR_EW_DT", "bf16")   # elementwise state dtype
ABLATE = os.environ.get("GLIFR_ABLATE", "")       # dev-only timing bisect

_DT = {"f32": mybir.dt.float32, "bf16": mybir.dt.bfloat16}
_NP = {"f32": np.float32, "bf16": ml_dtypes.bfloat16}

_CACHE = {}


def _build(mm_s, ew_s):
    mm = _DT[mm_s]
    ew = _DT[ew_s]
    f32 = mybir.dt.float32
    Act = mybir.ActivationFunctionType

    nc = bacc.Bacc("TRN2", target_bir_lowering=False, debug=False,
                   num_devices=N_CORES)

    # ---- DRAM parameters (per-core) ----
    d_xT = nc.dram_tensor("xT", [KCI, 128, T, BC], mm, kind="ExternalInput")
    d_win = nc.dram_tensor("w_in", [KCI, 128, HID], mm, kind="ExternalInput")
    d_wlat = nc.dram_tensor("w_lat", [J, 128, HID], mm, kind="ExternalInput")
    d_wout = nc.dram_tensor("w_out", [J, 128, OUT], mm, kind="ExternalInput")
    # all small elementwise constants packed into two tensors (one DMA each):
    # cb = [cP | cQ | cS | cC2 | d1_0] along the free axis, cf = [biasx|bout]
    d_cb = nc.dram_tensor("cb", [128, 3 * NA * J * BC + 3 * J * BC], ew,
                          kind="ExternalInput")
    d_cf = nc.dram_tensor("cf", [128, J], f32, kind="ExternalInput")
    d_out = nc.dram_tensor("outT", [OC, 128, T, BC], f32, kind="ExternalOutput")

    HB = TB // 2   # lateral half-block = 10 steps

    with tile.TileContext(nc) as tc:
        with (
            tc.tile_pool(name="weights", bufs=1) as wpool,
            tc.tile_pool(name="state", bufs=1) as spool,
            tc.tile_pool(name="ew", bufs=2) as epool,
            tc.tile_pool(name="synp", bufs=2) as synpool,
            tc.tile_pool(name="ost", bufs=2) as opool,
            tc.tile_pool(name="ps_xp", bufs=1, space="PSUM") as psxp,
            tc.tile_pool(name="ps_lat", bufs=1, space="PSUM") as pslat,
            tc.tile_pool(name="ps_ro", bufs=1, space="PSUM") as psro,
        ):
            # ---- persistent tiles ----
            t_win = wpool.tile([128, KCI, HID], mm, tag="win")
            t_wlat = wpool.tile([128, J, HID], mm, tag="wlat")
            t_wout = wpool.tile([128, J, OUT], mm, tag="wout")
            # packed constants (one DMA each); shaped views below
            NAJB = NA * J * BC
            JB = J * BC
            t_cb = wpool.tile([128, 3 * NAJB + 3 * JB], ew, tag="cb")
            t_cf = wpool.tile([128, J], f32, tag="cf")
            def cb3(i):
                return t_cb[:, i * NAJB:(i + 1) * NAJB].rearrange(
                    "p (a j b) -> p a j b", a=NA, j=J)

            def cb1(i):
                return t_cb[:, 3 * NAJB + i * JB:3 * NAJB + (i + 1) * JB] \
                    .rearrange("p (j b) -> p j b", j=J)

            ap_cP, ap_cQ, ap_cS = cb3(0), cb3(1), cb3(2)
            ap_cC2, ap_d1init, ap_cbx = cb1(0), cb1(1), cb1(2)

            def ap_biasx(j):
                return t_cf[:, j:j + 1]

            # x split head/rest: tile-granular deps mean block-0 matmuls
            # must not share a tile with the bulk transfer
            t_xTh = wpool.tile([128, KCI, TB, BC], mm, tag="xTh")
            t_xTr = wpool.tile([128, KCI, T - TB, BC], mm, tag="xTr")
            # Block 0 has no lateral input (delay buffer is zeros), so its
            # synaptic drive is just c1*x@W_in - sg*thresh, staged here. For
            # blocks >= 1 the x-projection accumulates directly into the
            # lateral PSUM banks (emit_group) and never lands in SBUF
            # separately.
            t_xsyn0a = wpool.tile([128, J, 4, BC], mybir.dt.bfloat16,
                                  tag="xsyn0a")
            t_xsyn0b = wpool.tile([128, J, TB - 4, BC], mybir.dt.bfloat16,
                                  tag="xsyn0b")

            # F_buf slot s holds firing(s-1); slot 0 = zeros
            t_F = spool.tile([128, J, T + 1, BC], mm, tag="F")
            t_A = spool.tile([128, NA, J, BC], ew, tag="A")
            t_Aq = spool.tile([128, NA, J, BC], ew, tag="Aq")
            t_Bst = spool.tile([128, NA, J, BC], ew, tag="Bst")
            t_vs = [spool.tile([128, J, BC], ew, tag=f"vs{i}", name=f"vs{i}")
                    for i in range(2)]
            t_D = [spool.tile([128, J, BC], ew, tag=f"D{i}", name=f"D{i}")
                    for i in range(2)]

            # ---- act-table preload + input DMAs ----
            # Dummy sigmoid on a zeroed scratch forces the 1.3us activation
            # table load to overlap the input DMAs instead of serializing
            # before the first real activation.
            t_scr = wpool.tile([128, 1], ew, tag="scr")
            nc.vector.memset(t_scr[:], 0.0)
            nc.scalar.activation(out=t_scr[:], in_=t_scr[:], func=Act.Sigmoid)
            # DMA order mirrors first-use time; the model serializes all
            # transfers, so the tiny constants must not queue behind bulk.
            nc.sync.dma_start(out=t_cb[:], in_=d_cb.ap())
            nc.sync.dma_start(out=t_cf[:], in_=d_cf.ap())
            nc.sync.dma_start(out=t_win[:],
                              in_=d_win.ap().rearrange("k p h -> p k h"))
            nc.scalar.dma_start(out=t_xTh[:],
                                in_=d_xT.ap()[:, :, 0:TB, :]
                                    .rearrange("k p t b -> p k t b"))
            nc.sync.dma_start(out=t_wlat[:],
                              in_=d_wlat.ap().rearrange("k p h -> p k h"))
            nc.sync.dma_start(out=t_xTr[:],
                              in_=d_xT.ap()[:, :, TB:T, :]
                                  .rearrange("k p t b -> p k t b"))
            nc.sync.dma_start(out=t_wout[:],
                              in_=d_wout.ap().rearrange("k p o -> p k o"))

            # ---- state init ----
            nc.vector.memset(t_Aq[:], 0.0)
            nc.vector.memset(t_Bst[:], 0.0)   # u(0)=0 makes any Bst(-1) ok
            nc.vector.memset(t_F[:, :, 0, :], 0.0)

            # block-0 xproj: two chunks, each all-j in one psum tile and
            # ONE bias-free ACT copy (the -sg*thresh bias is applied by an
            # extra Pool add in block 0's D-chain and folded into D(-1)).
            XCHUNKS0 = [(0, 4), (4, 16)]

            def emit_xproj_chunk(tci, dst):
                lo, ln = XCHUNKS0[tci]
                if "no_mm" in ABLATE:
                    nc.gpsimd.memset(dst[:], 0.0)
                    return
                ps = psxp.tile([128, J, 16, BC], f32, tag="xp0")
                for j in range(J):
                    for kc in range(KCI):
                        nc.tensor.matmul(
                            out=ps[:, j, 0:ln, :],
                            lhsT=t_win[:, kc, j * 128:(j + 1) * 128],
                            rhs=t_xTh[:, kc, lo:lo + ln, :],
                            start=(kc == 0), stop=(kc == KCI - 1))
                nc.scalar.activation(
                    out=dst[:], in_=ps[:, :, 0:ln, :], func=Act.Identity,
                    scale=1.0)

            def emit_group(k, ps, j, a, b):
                """syn psum group for block k (>=1), chunk j, steps [a,b) of
                the block: xproj matmuls open the accumulation group, the
                lateral matmuls accumulate on top. The lateral input is
                firing(k*TB+a-20 .. ), i.e. F slots (k-1)*TB+1+a, so the
                group is runnable once sigma(k*TB+a+b-TB-? ) -- in practice
                once sigma((k-1)*TB + b - 1) of the previous block fired."""
                if "no_mm" in ABLATE:
                    return
                ln = b - a
                out = ps[:, j, a * BC:b * BC].rearrange(
                    "p (t b) -> p t b", t=ln)
                x0 = (k - 1) * TB + a   # t_xTr starts at t=TB; k >= 1 here
                for kc in range(KCI):
                    nc.tensor.matmul(
                        out=out,
                        lhsT=t_win[:, kc, j * 128:(j + 1) * 128],
                        rhs=t_xTr[:, kc, x0:x0 + ln, :],
                        start=(kc == 0),
                        stop=(kc == KCI - 1 and "no_lat" in ABLATE))
                if "no_lat" in ABLATE:
                    return
                s0 = (k - 1) * TB + 1 + a
                for kc in range(J):
                    nc.tensor.matmul(
                        out=out,
                        lhsT=t_wlat[:, kc, j * 128:(j + 1) * 128],
                        rhs=t_F[:, kc, s0:s0 + ln, :],
                        start=False, stop=(kc == J - 1))

            def emit_syn_copy(k, ps, syn, j, a, b, pin_t=None):
                """syn_sb[j, a:b] = psum (xproj + lateral) + bias, one ACT
                copy; the per-hidden bias rides the activation's bias port.
                Pinned after the sigmoid one step past the group's last
                lateral input so it never stalls the sigmoid cadence."""
                if "no_mm" in ABLATE:
                    nc.gpsimd.memset(syn[:, j, a:b, :], 0.0)
                    return
                ci = nc.scalar.activation(
                    out=syn[:, j, a:b, :],
                    in_=ps[:, j, a * BC:b * BC].rearrange(
                        "p (t b) -> p t b", t=b - a),
                    func=Act.Identity, bias=ap_biasx(j), scale=1.0)
                pin_after_sigma(ci, pin_t if pin_t is not None
                                else (k - 1) * TB + b - 1)

            def emit_ro(k, deferred, defer_map=None, pieces=4):
                """readout for block k in `pieces` t-slices; each slice is
                4oc x 8kc matmuls + ONE bias-free store copy (all oc) + ONE
                DMA. b_out is added host-side after the gather. With
                defer_map, slice q's work goes to defer_map[q] so only the
                last slice trails the final sigmoid."""
                if "no_mm" in ABLATE or "no_ro" in ABLATE:
                    return
                QL = TB // pieces
                ps = psro.tile([128, OC, 256], f32, tag="ro")
                for q in range(pieces):
                    tgt = deferred if defer_map is None else defer_map[q]
                    s0 = k * TB + 1 + q * QL
                    for oc in range(OC):
                        for kc in range(J):
                            tgt.append(
                                lambda oc=oc, kc=kc, q=q, s0=s0, ps=ps:
                                nc.tensor.matmul(
                                    out=ps[:, oc,
                                           q * QL * BC:(q + 1) * QL * BC]
                                    .rearrange("p (t b) -> p t b", t=QL),
                                    lhsT=t_wout[:, kc,
                                                oc * 128:(oc + 1) * 128],
                                    rhs=t_F[:, kc, s0:s0 + QL, :],
                                    start=(kc == 0), stop=(kc == J - 1)))

                    def flush(q=q, ps=ps, s0=s0):
                        ot = opool.tile([128, OC, QL, BC], f32, tag="ost")
                        ci = nc.scalar.activation(
                            out=ot[:],
                            in_=ps[:, :, q * QL * BC:(q + 1) * QL * BC]
                            .rearrange("p o (t b) -> p o t b", t=QL),
                            func=Act.Identity, scale=1.0)
                        pin_after_sigma(ci, s0 + QL - 1)
                        nc.sync.dma_start(
                            out=d_out.ap()[:, :,
                                           k * TB + q * QL:
                                           k * TB + (q + 1) * QL, :]
                            .rearrange("o p t b -> p o t b"),
                            in_=ot[:])
                    tgt.append(flush)

            sig_insts = {}

            def pin_after_sigma(inst, t):
                """Order `inst` after sigma(t): keeps the scheduler from
                slotting PSUM-copy ops into the ACT stream before their
                matmul group has finished, which stalls later sigmoids."""
                si = sig_insts.get(min(t, T - 1))
                if si is not None and inst is not None:
                    add_dep_helper(inst.ins, si.ins,
                                   reason="pin copy after sigma")

            def emit_ew_step(t, syn, syn_funcs):
                """B-form recurrence step; reads F slot t, writes slot t+1.

                Critical path after sigma(t-1): mb2 -> msum -> vs -> sigma(t).
                Everything else overlaps the ScalarE sigmoid round-trip; the
                d1 update runs on the GPSIMD engine. vs and d1 are
                double-buffered (t%2) to break cross-engine WAR stalls.
                """
                if "no_ew" in ABLATE:
                    return
                cur, prv = t % 2, (t + 1) % 2
                u2 = t_F[:, :, t, :].unsqueeze(1) \
                    .broadcast_to([128, NA, J, BC])
                # critical: vs(t) = u(t)*(B0+B1)(t-1) + D(t-1)
                mb2 = epool.tile([128, NA, J, BC], ew, tag="mb2")
                msum = epool.tile([128, J, BC], ew, tag="msum")
                with tc.high_priority(offset=40):
                    nc.vector.tensor_mul(out=mb2[:], in0=u2, in1=t_Bst[:])
                    nc.vector.tensor_add(out=msum[:], in0=mb2[:, 0],
                                         in1=mb2[:, 1])
                    nc.vector.tensor_add(out=t_vs[cur][:], in0=msum[:],
                                         in1=t_D[prv][:])
                    if "no_sigma" not in ABLATE:
                        sig_insts[t] = nc.scalar.activation(
                            out=t_F[:, :, t + 1, :],
                            in_=t_vs[cur][:], func=Act.Sigmoid)
                # state updates (overlap sigma): A(t) = Aq(t-1) + mb2
                nc.vector.tensor_add(out=t_A[:], in0=t_Aq[:], in1=mb2[:])
                nc.vector.tensor_mul(out=t_Aq[:], in0=t_A[:], in1=ap_cQ)
                qa = epool.tile([128, J, BC], ew, tag="qa")
                nc.vector.tensor_add(out=qa[:], in0=t_Aq[:, 0], in1=t_Aq[:, 1])
                # B(t) = p*A(t) + s'   (on the step loop -> keep on DVE)
                bp = epool.tile([128, NA, J, BC], ew, tag="bp")
                nc.vector.tensor_mul(out=bp[:], in0=t_A[:], in1=ap_cP)
                nc.vector.tensor_add(out=t_Bst[:], in0=bp[:], in1=ap_cS)
                # D(t) = qa + syn'(t+1) + c2*vs(t) on Pool. d1 = qa+syn does
                # not depend on vs, so only cv -> D trails the vs write; D
                # lands ~vs+650 instead of ~vs+1100 (3-op serial chain).
                sc = epool.tile([128, J, BC], ew, tag="scv")
                cv = epool.tile([128, J, BC], ew, tag="cv")
                if t + 1 < T:
                    nxt = syn_funcs[(t + 1) // TB]
                    # D = (c2*vs + syn'(t+1)) + qa, assembled so only the
                    # final add trails qa (the last-arriving operand): the
                    # qa -> D path is one Pool op instead of two.
                    nc.gpsimd.tensor_mul(out=cv[:], in0=t_vs[cur][:],
                                         in1=ap_cC2)
                    nc.gpsimd.tensor_add(out=sc[:], in0=cv[:], in1=nxt(t + 1))
                    if t + 1 < TB:
                        # block-0 syn slices are bias-free (batched copies):
                        # apply -sg*thresh here
                        sb = epool.tile([128, J, BC], ew, tag="scb")
                        nc.gpsimd.tensor_add(out=sb[:], in0=sc[:], in1=ap_cbx)
                        sc = sb
                    nc.gpsimd.tensor_add(out=t_D[cur][:], in0=sc[:],
                                         in1=qa[:])

            # ---------- main schedule ----------
            # Block-0 input projection upfront. D(-1) init only needs the
            # first 4-step chunk (separate tile -> no false dep on chunk 2).
            emit_xproj_chunk(0, t_xsyn0a)
            nc.gpsimd.tensor_add(out=t_D[1][:], in0=ap_d1init,
                                 in1=t_xsyn0a[:, :, 0, :])
            emit_xproj_chunk(1, t_xsyn0b)

            def xsyn0_slice(t):
                if t < 4:
                    return t_xsyn0a[:, :, t, :]
                return t_xsyn0b[:, :, t - 4, :]

            def synsb_slice(syn):
                def f(t):
                    return syn[:, :, t % TB, :]
                return f


            syn_funcs = {0: xsyn0_slice}     # block 0 reads xsyn0 directly
            ps_next = None
            syn_next = None
            defC = []   # this block's half-1 syn groups (data complete at
                        # the preceding block boundary); popped steps 0..8
            for k in range(NBLK):
                # defA: popped during EW steps 0..8 after defC: block k-1's
                #   readout.
                # defB: popped during EW steps 10..18: block k+1 syn psum
                #   groups, half 0 (xproj + lateral, consuming this block's
                #   first-half firing as it appears) + their syn copies.
                defA, defB, defQ = list(defC), [], []
                defC = []
                if k >= 1:
                    emit_ro(k - 1, defA)
                if k + 1 < NBLK:
                    ps_next = pslat.tile([128, J, 256], f32, tag="lat")
                    syn_next = synpool.tile([128, J, TB, BC], ew, tag="syn_sb")
                    syn_funcs[k + 1] = synsb_slice(syn_next)
                    for j in range(J):
                        # half 0 (steps 0..9 of block k+1): lateral input is
                        # firing from this block's steps 1..10 -> runnable
                        # mid-block; popped steps 10..14.
                        defB.append(lambda j=j, ps=ps_next, kk=k + 1:
                                    emit_group(kk, ps, j, 0, HB))
                        defB.append(lambda j=j, ps=ps_next, sy=syn_next,
                                    kk=k + 1, pt=k * TB + 11 + j // 2:
                                    emit_syn_copy(kk, ps, sy, j, 0, HB,
                                                  pin_t=pt))
                        # quarter q2 (steps 10..14): needs sigma(step 14) of
                        # this block -> drains during this block's tail
                        # (groups only; the copy waits for q3).
                        defQ.append(lambda j=j, ps=ps_next, kk=k + 1:
                                    emit_group(kk, ps, j, HB, HB + 5))
                        # quarter q3 (steps 15..19): needs this block's last
                        # sigmoid; popped during EW(k+1) steps 0..8. One
                        # copy then covers the whole half [10,20), landing
                        # before E(step 9) consumes syn(10).
                        defC.append(lambda j=j, ps=ps_next, kk=k + 1:
                                    emit_group(kk, ps, j, HB + 5, TB))
                        defC.append(lambda j=j, ps=ps_next, sy=syn_next,
                                    kk=k + 1, pt=(k + 1) * TB + j + 1:
                                    emit_syn_copy(kk, ps, sy, j, HB, TB,
                                                  pin_t=pt))
                if k == NBLK - 1:
                    ro_tail = []
                    # quarters q0/q1 during steps 10-14, q2 during 15-18,
                    # q3 (needs the last sigmoid) in the tail
                    emit_ro(NBLK - 1, None, pieces=4,
                            defer_map={0: defB, 1: defB, 2: defQ,
                                       3: ro_tail})

                perA = max(1, (len(defA) + 8) // 9)
                perB = max(1, (len(defB) + 4) // 5)
                perQ = max(1, (len(defQ) + 3) // 4)
                for li, t in enumerate(range(k * TB, (k + 1) * TB)):
                    emit_ew_step(t, syn_funcs[k], syn_funcs)
                    if li < 10:
                        pend, per = defA, perA
                    elif li < 15:
                        pend, per = defB, perB
                    else:
                        pend, per = defQ, perQ
                    for _ in range(per):
                        if pend:
                            pend.pop(0)()
                for fn in defA + defB + defQ:
                    fn()

            # final readout tail (half 1 + stores; half 0 ran in EW(9))
            for fn in ro_tail:
                fn()

    nc.compile()
    return nc


def _sigmoid(x):
    return 1.0 / (1.0 + np.exp(-x))


def _prep(inputs, mm_s, ew_s):
    mmn = _NP[mm_s]
    ewn = _NP[ew_s]
    f32 = np.float32

    x = np.asarray(inputs["x"], f32)
    W_in = np.asarray(inputs["W_in"], f32)
    W_lat = np.asarray(inputs["W_lat"], f32)
    thresh = np.asarray(inputs["thresh"], f32)[0]
    trans_k_m = np.asarray(inputs["trans_k_m"], f32)[0]
    trans_asc_k = np.asarray(inputs["trans_asc_k"], f32)[:, 0, :]
    asc_amp = np.asarray(inputs["asc_amp"], f32)[:, 0, :]
    trans_asc_r = np.asarray(inputs["trans_asc_r"], f32)[:, 0, :]
    W_out = np.asarray(inputs["W_out"], f32)
    b_out = np.asarray(inputs["b_out"], f32)

    sg = _sigmoid(trans_k_m).astype(f32)
    c1 = (R_MEM * sg).astype(f32)
    c2 = (1.0 - sg).astype(f32)
    dka = _sigmoid(trans_asc_k).astype(f32)
    r_a = (1.0 - 2.0 * _sigmoid(trans_asc_r)).astype(f32)
    p_a = (r_a * dka).astype(f32)
    q_a = (1.0 - dka).astype(f32)
    s_a = (c1[None] * dka * asc_amp).astype(f32)
    bias_h = (-sg * thresh).astype(f32)

    w_in = (W_in * c1[None, :]).astype(mmn).reshape(KCI, 128, HID)
    w_lat = (W_lat * c1[None, :]).astype(mmn).reshape(J, 128, HID)
    w_out = np.ascontiguousarray(W_out.T).astype(mmn).reshape(J, 128, OUT)

    def hb(coef_ah):  # [NA,H] -> [128, NA, J, BC]
        a = coef_ah.reshape(NA, J, 128).transpose(2, 0, 1)
        return np.broadcast_to(a[..., None], (128, NA, J, BC)).astype(ewn).copy()

    def hb1(coef_h):  # [H] -> [128, J, BC]
        a = coef_h.reshape(J, 128).T
        return np.broadcast_to(a[..., None], (128, J, BC)).astype(ewn).copy()

    cP, cQ, cS = hb(p_a), hb(q_a), hb(s_a)
    cC2 = hb1(c2)
    # block-0 syn slices carry no bias, so D(-1) folds it in: -(c2+sg)*th
    d1_0 = hb1((-(c2 + sg) * thresh).astype(f32))
    cbx = hb1(bias_h)
    bias_x = np.ascontiguousarray(bias_h.reshape(J, 128).T).astype(f32)
    b_outT = np.ascontiguousarray(b_out.reshape(OC, 128).T).astype(f32)

    # packed constants: cb = [cP|cQ|cS|cC2|d1_0|cbx], cf = biasx
    cb = np.concatenate(
        [a.reshape(128, -1) for a in (cP, cQ, cS, cC2, d1_0, cbx)],
        axis=1).astype(ewn)
    cf = bias_x

    in_maps = []
    for c in range(N_CORES):
        xc = x[c * BC:(c + 1) * BC]                    # [8, 200, 512]
        xT = np.ascontiguousarray(xc.transpose(2, 1, 0)).astype(mmn) \
            .reshape(KCI, 128, T, BC)
        in_maps.append({
            "xT": xT, "w_in": w_in, "w_lat": w_lat, "w_out": w_out,
            "cb": cb, "cf": cf,
        })
    return in_maps


def _get_nc():
    key = (MM_DT_S, EW_DT_S, ABLATE)
    if key not in _CACHE:
        _CACHE[key] = _build(MM_DT_S, EW_DT_S)
    return _CACHE[key]


def kernel(**inputs) -> np.ndarray:
    nc = _get_nc()
    in_maps = _prep(inputs, MM_DT_S, EW_DT_S)
    try:
        res = run_bass_kernel_spmd(nc, in_maps, list(range(N_CORES)))
    except Exception:
        # transient NRT device errors have been observed through the axon
        # tunnel; one retry normally succeeds
        import time as _time
        _time.sleep(2.0)
        res = run_bass_kernel_spmd(nc, in_maps, list(range(N_CORES)))
    out = np.empty((B, T, OUT), np.float32)
    for c in range(N_CORES):
        r = res.results[c]["outT"]                     # [OC, 128, T, BC]
        out[c * BC:(c + 1) * BC] = r.transpose(3, 2, 0, 1).reshape(BC, T, OUT)
    out += np.asarray(inputs["b_out"], np.float32)  # bias applied host-side
    return out

